# revision 10
# baseline (speedup 1.0000x reference)
"""Causal multi-head attention block (B=2, T=2048, C=1024, H=16) on 8 TRN2
NeuronCores.

Sharding: 2D tensor parallel — core r owns batch b = r//4 and head quad
g = r%4 (heads 4g..4g+3, feature slice [256g, 256g+256)). Each core
projects q/k/v for its 256 features over its batch's 2048 tokens
(x^T replicated per batch), runs causal attention for its 4 heads, then
computes a PARTIAL output projection out_partial = Wo[:, 256g:256g+256]
@ y_local — full 1024 output rows, contracting only the local features.
The 4 cores of a batch ReduceScatter(add) the partials so core g ends
with output rows [256g, 256g+256) — 3 MB of wire per core instead of
the ~7 MB an 8-way y-AllGather moves, and the O-projection matmuls are
collective-independent so they act as PE filler instead of tail work.

On-device everything is feature-major (transposed) so the TensorEngine
contraction axis sits on SBUF partitions and the softmax denominator
arrives via a ones-column appended to V:

  qT/kT/vT [128, 2, 2048] = W_shard @ x^T        (2 feature tiles)
  ST tile [128k, 512q] = kT[64h slice].T @ qT    (contract d=64)
  causal mask: add a -1e9 strictly-lower-triangular matrix into the St
      PSUM accumulation group via matmul(ident, mneg) on diagonal blocks
  PT = exp(ST * 1/sqrt(d))                       (no max-subtraction:
                                                  logits ~N(0,1))
  yT [65, 512] += [v | 1].T @ PT                 (row 64 = softmax denom)
  ych[*, 512]  = yT[0:64] * bcast(recip(yT[64])) (written per head into
                                                  a [128, 2, 512] tile)
  po [128rt, 512] += WoT[ci, rt] @ ych[:, ci]    (8 row tiles × 2 ci)

Performance structure (from perfetto/HAM analysis of the v1 kernel):
- k-tiles are processed in pairs sharing one 2-bank PSUM tile so each
  EXP covers ~1024 columns (ACT has ~352-cycle fixed cost/instruction).
- Score matmuls run one pair AHEAD of the PV matmuls of the previous
  pair, so the PE never waits on the exp of the pair it just scored.
- The PE idle-activity throttle (HAM) halves the clock after ~3.4us of
  idle; projection chunks and partial-O groups are interleaved into the
  attention stream as real filler to keep the matmul stream dense.
- ReduceScatter chunks fire per 512-token block as soon as the block's
  partial-O lands in DRAM; only the last block's RS is tail-exposed.

Inputs are bf16 (host-side cast); accumulation is f32 in PSUM; the
output shard is written bf16 and upcast to f32 on the host.
"""

import numpy as np
import ml_dtypes

import concourse.bacc as bacc
import concourse.mybir as mybir
import concourse.tile as tile
from concourse.bass_utils import run_bass_kernel_spmd
from concourse.masks import make_identity

N_CORES = 8
B, T, C, H = 2, 2048, 1024, 16
D = 64                # head dim
GR = 4                # head-group cores per batch
HL = 4                # heads per core
DL = HL * D           # local feature dim = 256
MT = DL // 128        # feature tiles per core = 2
TL = T                # local tokens per core = its batch's 2048
P = 128
NCH = C // P          # 8 contraction chunks for q/k/v projections
QCH = 512             # q-chunk / token chunk
NQC = TL // QCH       # 4 chunks
NKT = TL // P         # 16 k-tiles
NRT = C // P          # 8 output row tiles for partial O-proj
SCALE = 1.0 / np.sqrt(D)

BF = mybir.dt.bfloat16
F32 = mybir.dt.float32
AF = mybir.ActivationFunctionType

RGROUPS = [[0, 1, 2, 3], [4, 5, 6, 7]]


def build_graph():
    nc = bacc.Bacc("TRN2", target_bir_lowering=False, debug=False)

    xT = nc.dram_tensor("xT", [C, TL], BF, kind="ExternalInput")
    # qkv shards [p, w, ci, m(256)] then woT [p, ci2, rt*128]
    WQKV = 3 * NCH * DL
    WO = MT * C
    wall = nc.dram_tensor("wall", [P, WQKV + WO], BF, kind="ExternalInput")
    out = nc.dram_tensor("out", [DL, TL], BF, kind="ExternalOutput")

    with tile.TileContext(nc) as tc:
        with (
            tc.tile_pool(name="sb", bufs=1) as sb,
            tc.tile_pool(name="ps", bufs=1, space="PSUM") as ps,
            tc.tile_pool(name="dram", bufs=1, space="DRAM") as dram,
        ):
            # ---- loads ----
            w_sb = sb.tile([P, WQKV + WO], BF, name="w_sb")
            WCOLS = WQKV + WO
            for pc in range(8):
                csl = slice(pc * (WCOLS // 8), (pc + 1) * (WCOLS // 8))
                nc.sync.dma_start(w_sb[:, csl], wall[:, csl])
            w3 = w_sb[:, 0:WQKV].rearrange("p (w a m) -> p w a m", w=3, a=NCH)
            wq_sb, wk_sb, wv_sb = (w3[:, i] for i in range(3))
            wo_sb = w_sb[:, WQKV:].rearrange("p (a r) -> p a r", a=MT)

            ident = sb.tile([P, P], BF, name="ident")
            make_identity(nc, ident)
            # strictly-lower-triangular -1e9 (k > q) for diagonal blocks
            mneg = sb.tile([P, P], BF, name="mneg")
            nc.gpsimd.memset(mneg[:], 0.0)
            nc.gpsimd.affine_select(
                out=mneg[:], in_=mneg[:],
                compare_op=mybir.AluOpType.is_ge,
                fill=-1e9, base=0, channel_multiplier=-1, pattern=[[1, P]],
            )
            wsrc = sb.tile([P, QCH], BF, name="wsrc")
            nc.vector.memset(wsrc[:], 0.5)

            def keepwarm(n):
                for _ in range(n):
                    wdst = ps.tile([P, QCH], F32, tag="st", bufs=2,
                                   name="wdst")
                    nc.tensor.matmul(wdst[:], ident[:], wsrc[:],
                                     start=True, stop=True)

            keepwarm(18)

            qT_sb = sb.tile([P, MT, TL], BF, name="qT_sb")
            kT_sb = sb.tile([P, MT, TL], BF, name="kT_sb")
            vT_sb = sb.tile([P, MT, TL], BF, name="vT_sb")
            # v natural layout, per 128-token tile per head: [64 v | 1]
            v_sb = sb.tile([P, NKT, HL, D + 1], BF, name="v_sb")
            nc.gpsimd.memset(v_sb[:], 1.0)

            # partial-O DRAM staging + RS outputs
            po_dram = [
                dram.tile([C, QCH], BF, name=f"po_dram{c}")
                for c in range(NQC)
            ]
            rs_out = [
                dram.tile([DL, QCH], BF, name=f"rs_out{c}")
                for c in range(NQC)
            ]

            with tc.tile_pool(name="xp", bufs=1) as xp:
                xT_sb = xp.tile([P, NCH, TL], BF, name="xT_sb")
                # chunk 0 fast (8 small DMAs on scalar), rest batched
                for ci in range(NCH):
                    nc.scalar.dma_start(
                        xT_sb[:, ci, 0:QCH], xT[ci * P:(ci + 1) * P, 0:QCH]
                    )
                for s0, s1 in ((QCH, 2 * QCH), (2 * QCH, TL)):
                    for ci in range(NCH):
                        nc.gpsimd.dma_start(
                            xT_sb[:, ci, s0:s1], xT[ci * P:(ci + 1) * P, s0:s1]
                        )

                def proj_group(tch, wsb, mt, dst):
                    tsl = slice(tch * QCH, (tch + 1) * QCH)
                    pj = ps.tile([P, QCH], F32, tag="st", bufs=2, name="pj")
                    for ci in range(NCH):
                        nc.tensor.matmul(
                            pj[:], wsb[:, ci, mt * P:(mt + 1) * P],
                            xT_sb[:, ci, tsl],
                            start=(ci == 0), stop=(ci == NCH - 1),
                        )
                    nc.vector.tensor_copy(dst[:, mt, tsl], pj[:])

                def vtrans(t16, mt):
                    tr = ps.tile([P, P], BF, tag="st", bufs=2, name="tr")
                    nc.tensor.transpose(
                        tr[:], vT_sb[:, mt, t16 * P:(t16 + 1) * P], ident[:]
                    )
                    out_ap = v_sb[:, t16, 2 * mt:2 * mt + 2, 0:D]
                    in_ap = tr[:].rearrange("p (h x) -> p h x", h=2)
                    nc.vector.tensor_copy(out_ap, in_ap)

                ych_tiles = {}

                def attn_head(jq, h, lookahead=True):
                    """Score+exp+PV for one head of one 512-token block.

                    Scores for pair p+1 are issued before the PVs of
                    pair p, so the PE isn't gated on exp(p)."""
                    mt, hh = h // 2, h % 2
                    rsl = slice(hh * D, (hh + 1) * D)
                    q0 = jq * QCH
                    yt = ps.tile([D + 1, QCH], F32, tag="yt", bufs=2,
                                 name="yt")
                    nkt = 4 * jq + 4
                    npr = nkt // 2

                    def scores(pr):
                        st = ps.tile([P, 2 * QCH], F32, tag="st", bufs=2,
                                     name="st")
                        pt = sb.tile([P, 2 * QCH], BF, tag="pt", bufs=4,
                                     name="pt")
                        for half in range(2):
                            kt = 2 * pr + half
                            k0 = kt * P
                            i = kt - 4 * jq
                            qv = max(i, 0) * P
                            ssl = slice(half * QCH + qv, (half + 1) * QCH)
                            nc.tensor.matmul(
                                st[:, ssl],
                                kT_sb[rsl, mt, k0:k0 + P],
                                qT_sb[rsl, mt, q0 + qv:q0 + QCH],
                                start=True, stop=(i < 0),
                            )
                            if i >= 0:
                                nc.tensor.matmul(
                                    st[:, half * QCH + qv:
                                       half * QCH + qv + P],
                                    ident[:], mneg[:],
                                    start=False, stop=True,
                                )
                        qv0 = max(2 * pr - 4 * jq, 0) * P
                        nc.scalar.activation(
                            pt[:, qv0:], st[:, qv0:], AF.Exp,
                            scale=float(SCALE)
                        )
                        return pt

                    def pv(pr, pt):
                        for half in range(2):
                            kt = 2 * pr + half
                            qv = max(kt - 4 * jq, 0) * P
                            nc.tensor.matmul(
                                yt[:, qv:QCH],
                                v_sb[:, kt, h, :],
                                pt[:, half * QCH + qv:(half + 1) * QCH],
                                start=(kt == 0), stop=(kt == nkt - 1),
                            )

                    if lookahead:
                        pts = [scores(0)]
                        for pr in range(npr):
                            if pr + 1 < npr:
                                pts.append(scores(pr + 1))
                            pv(pr, pts[pr])
                    else:
                        for pr in range(npr):
                            pv(pr, scores(pr))

                    den = sb.tile([1, QCH], F32, tag="den", bufs=4,
                                  name="den")
                    nc.vector.tensor_copy(den[:], yt[D:D + 1, :])
                    return yt, den

                def attn_evict(jq, h, yt, den):
                    mt, hh = h // 2, h % 2
                    bc = sb.tile([D, QCH], F32, tag="bc", bufs=3, name="bc")
                    nc.gpsimd.partition_broadcast(bc[:], den[:])
                    rcp = sb.tile([D, QCH], F32, tag="rcp", bufs=3,
                                  name="rcp")
                    scr = sb.tile([D, QCH], F32, tag="scr", bufs=3,
                                  name="scr")
                    nc.vector.reciprocal_approx_accurate(
                        rcp[:], bc[:], scratch=scr[:]
                    )
                    ych = ych_tiles[jq]
                    nc.vector.tensor_mul(
                        ych[hh * D:(hh + 1) * D, mt, :], yt[0:D, :], rcp[:]
                    )

                def po_group(c):
                    """Partial output projection for token chunk c:
                    po[1024, 512] = WoT_shard @ ych -> DRAM, then RS."""
                    ych = ych_tiles[c]
                    ob = sb.tile([P, NRT, QCH], BF, tag="ob", bufs=2,
                                 name="ob")
                    for rt in range(NRT):
                        po = ps.tile([P, QCH], F32, tag="st", bufs=2,
                                     name="po")
                        for ci in range(MT):
                            nc.tensor.matmul(
                                po[:], wo_sb[:, ci, rt * P:(rt + 1) * P],
                                ych[:, ci, :],
                                start=(ci == 0), stop=(ci == MT - 1),
                            )
                        nc.vector.tensor_copy(ob[:, rt, :], po[:])
                    dview = po_dram[c][:].rearrange("(r p) t -> p r t", p=P)
                    nc.sync.dma_start(dview, ob[:])
                    nc.gpsimd.collective_compute(
                        "ReduceScatter",
                        mybir.AluOpType.add,
                        replica_groups=RGROUPS,
                        ins=[po_dram[c][:]],
                        outs=[rs_out[c][:]],
                    )
                    nsp = 2 if c == NQC - 1 else 1
                    w = QCH // nsp
                    for s in range(nsp):
                        o0 = c * QCH + s * w
                        nc.gpsimd.dma_start(
                            out[:, o0:o0 + w],
                            rs_out[c][:, s * w:(s + 1) * w],
                        )

                # ---- filler queue: projection chunks issued JIT ----
                def proj_items(tchs):
                    items = []
                    for tch in tchs:
                        for wsb, dst in ((wq_sb, qT_sb), (wk_sb, kT_sb),
                                         (wv_sb, vT_sb)):
                            for mt in range(MT):
                                items.append((proj_group, (tch, wsb, mt,
                                                           dst)))
                        for t16 in range(tch * 4, tch * 4 + 4):
                            for mt in range(MT):
                                items.append((vtrans, (t16, mt)))
                    return items

                filler = []

                def pop_filler(n):
                    for _ in range(min(n, len(filler))):
                        fn, args = filler.pop(0)
                        fn(*args)

                # chunk 0 projections up front (attention needs them)
                for fn, args in proj_items([0]):
                    fn(*args)

                filler = proj_items([1, 2, 3])

                # ---- attention blocks ----
                for jq in range(NQC):
                    ych_tiles[jq] = sb.tile([P, MT, QCH], BF, tag="ych",
                                            bufs=2, name="ych")
                    for h in range(HL):
                        yt, den = attn_head(jq, h)
                        if jq < NQC - 1:
                            # chunk jq+1 projections as PE filler; spread
                            # across the 4 head gaps (14 items/chunk)
                            pop_filler(4 if h else 3)
                        elif h == 0:
                            pop_filler(99)
                        attn_evict(jq, h, yt, den)
                        if h == 1 and jq > 0:
                            po_group(jq - 1)
                    if jq == NQC - 1:
                        po_group(jq)
                keepwarm(6)

    nc.finalize()
    return nc


_GRAPH = None


def _get_graph():
    global _GRAPH
    if _GRAPH is None:
        _GRAPH = build_graph()
    return _GRAPH


def prepare_in_maps(x, Wq, Wk, Wv, Wo):
    x = np.asarray(x, np.float32)
    Wq = np.asarray(Wq, np.float32)
    Wk = np.asarray(Wk, np.float32)
    Wv = np.asarray(Wv, np.float32)
    Wo = np.asarray(Wo, np.float32)

    bf = ml_dtypes.bfloat16
    xTh = [
        np.ascontiguousarray(x[b].T).astype(bf) for b in range(B)
    ]  # [C, TL] each
    in_maps = []
    for r in range(N_CORES):
        b, g = r // GR, r % GR
        sl = slice(g * DL, (g + 1) * DL)
        wqkv = np.empty((P, 3, NCH, DL), np.float32)
        for w, W in enumerate((Wq, Wk, Wv)):
            # shard rows -> [in=(ci p), out] -> [p, ci, m]
            wqkv[:, w] = W[sl].T.reshape(NCH, P, DL).transpose(1, 0, 2)
        # woT[p, ci, rt*128]: out[rt block] += Wo[rt, ci block].T @ y[ci]
        wo = np.ascontiguousarray(Wo[:, sl].T)  # [DL, C] = lhsT
        woT = wo.reshape(MT, P, C).transpose(1, 0, 2)  # [p, ci, C]
        wall = np.concatenate(
            [wqkv.reshape(P, 3 * NCH * DL), woT.reshape(P, MT * C)], axis=1
        )
        in_maps.append({
            "xT": xTh[b],
            "wall": np.ascontiguousarray(wall).astype(bf),
        })
    return in_maps


def assemble_output(results):
    outT = np.empty((B, C, TL), np.float32)
    for r in range(N_CORES):
        b, g = r // GR, r % GR
        outT[b, g * DL:(g + 1) * DL] = np.asarray(
            results[r]["out"], np.float32
        )
    return np.ascontiguousarray(outT.transpose(0, 2, 1))  # [B, T, C]


def kernel(x, Wq, Wk, Wv, Wo):
    nc = _get_graph()
    in_maps = prepare_in_maps(x, Wq, Wk, Wv, Wo)
    res = run_bass_kernel_spmd(nc, in_maps, core_ids=list(range(N_CORES)))
    return assemble_output(res.results)


# revision 12
# speedup vs baseline: 1.0637x; 1.0637x over previous
"""Causal multi-head attention block (B=2, T=2048, C=1024, H=16) on 8 TRN2
NeuronCores.

Sharding: 2D tensor parallel — core r owns batch b = r//4 and head quad
g = r%4 (heads 4g..4g+3, feature slice [256g, 256g+256)). Each core
projects q/k/v for its 256 features over its batch's 2048 tokens
(x^T replicated per batch), runs causal attention for its 4 heads, then
computes a PARTIAL output projection out_partial = Wo[:, 256g:256g+256]
@ y_local — full 1024 output rows, contracting only the local features.
The 4 cores of a batch ReduceScatter(add) the partials so core g ends
with output rows [256g, 256g+256) — 3 MB of wire per core instead of
the ~7 MB an 8-way y-AllGather moves, and the O-projection matmuls are
collective-independent so they act as PE filler instead of tail work.

On-device everything is feature-major (transposed) so the TensorEngine
contraction axis sits on SBUF partitions and the softmax denominator
arrives via a ones-column appended to V:

  qT/kT/vT [128, 2, 2048] = W_shard @ x^T        (2 feature tiles)
  ST tile [128k, 512q] = kT[64h slice].T @ qT    (contract d=64)
  causal mask: add a -1e9 strictly-lower-triangular matrix into the St
      PSUM accumulation group via matmul(ident, mneg) on diagonal blocks
  PT = exp(ST * 1/sqrt(d))                       (no max-subtraction:
                                                  logits ~N(0,1))
  yT [65, 512] += [v | 1].T @ PT                 (row 64 = softmax denom)
  ych[*, 512]  = yT[0:64] * bcast(recip(yT[64])) (written per head into
                                                  a [128, 2, 512] tile)
  po [128rt, 512] += WoT[ci, rt] @ ych[:, ci]    (8 row tiles × 2 ci)

Performance structure (from perfetto/HAM analysis of the v1 kernel):
- k-tiles are processed in pairs sharing one 2-bank PSUM tile so each
  EXP covers ~1024 columns (ACT has ~352-cycle fixed cost/instruction).
- Score matmuls run one pair AHEAD of the PV matmuls of the previous
  pair, so the PE never waits on the exp of the pair it just scored.
- The PE idle-activity throttle (HAM) halves the clock after ~3.4us of
  idle; projection chunks and partial-O groups are interleaved into the
  attention stream as real filler to keep the matmul stream dense.
- ReduceScatter chunks fire per 512-token block as soon as the block's
  partial-O lands in DRAM; only the last block's RS is tail-exposed.

Inputs are bf16 (host-side cast); accumulation is f32 in PSUM; the
output shard is written bf16 and upcast to f32 on the host.
"""

import numpy as np
import ml_dtypes

import concourse.bacc as bacc
import concourse.mybir as mybir
import concourse.tile as tile
from concourse.bass_utils import run_bass_kernel_spmd
from concourse.masks import make_identity

N_CORES = 8
B, T, C, H = 2, 2048, 1024, 16
D = 64                # head dim
GR = 4                # head-group cores per batch
HL = 4                # heads per core
DL = HL * D           # local feature dim = 256
MT = DL // 128        # feature tiles per core = 2
TL = T                # local tokens per core = its batch's 2048
P = 128
NCH = C // P          # 8 contraction chunks for q/k/v projections
QCH = 512             # q-chunk / token chunk
NQC = TL // QCH       # 4 chunks
NKT = TL // P         # 16 k-tiles
NRT = C // P          # 8 output row tiles for partial O-proj
SCALE = 1.0 / np.sqrt(D)

BF = mybir.dt.bfloat16
F32 = mybir.dt.float32
AF = mybir.ActivationFunctionType

RGROUPS = [[0, 1, 2, 3], [4, 5, 6, 7]]


def build_graph():
    nc = bacc.Bacc("TRN2", target_bir_lowering=False, debug=False)

    xT = nc.dram_tensor("xT", [C, TL], BF, kind="ExternalInput")
    # qkv shards [p, w, ci, m(256)] then woT [p, ci2, rt*128]
    WQKV = 3 * NCH * DL
    WO = MT * C
    wall = nc.dram_tensor("wall", [P, WQKV + WO], BF, kind="ExternalInput")
    out = nc.dram_tensor("out", [DL, TL], BF, kind="ExternalOutput")

    with tile.TileContext(nc) as tc:
        with (
            tc.tile_pool(name="sb", bufs=1) as sb,
            tc.tile_pool(name="ps", bufs=1, space="PSUM") as ps,
            tc.tile_pool(name="dram", bufs=1, space="DRAM") as dram,
        ):
            # ---- loads ----
            w_sb = sb.tile([P, WQKV + WO], BF, name="w_sb")
            WCOLS = WQKV + WO
            for pc in range(8):
                csl = slice(pc * (WCOLS // 8), (pc + 1) * (WCOLS // 8))
                nc.sync.dma_start(w_sb[:, csl], wall[:, csl])
            w3 = w_sb[:, 0:WQKV].rearrange("p (w a m) -> p w a m", w=3, a=NCH)
            wq_sb, wk_sb, wv_sb = (w3[:, i] for i in range(3))
            wo_sb = w_sb[:, WQKV:].rearrange("p (a r) -> p a r", a=MT)

            ident = sb.tile([P, P], BF, name="ident")
            make_identity(nc, ident)
            # strictly-lower-triangular -1e9 (k > q) for diagonal blocks
            mneg = sb.tile([P, P], BF, name="mneg")
            nc.gpsimd.memset(mneg[:], 0.0)
            nc.gpsimd.affine_select(
                out=mneg[:], in_=mneg[:],
                compare_op=mybir.AluOpType.is_ge,
                fill=-1e9, base=0, channel_multiplier=-1, pattern=[[1, P]],
            )
            wsrc = sb.tile([P, QCH], BF, name="wsrc")
            nc.vector.memset(wsrc[:], 0.5)

            def keepwarm(n):
                for _ in range(n):
                    wdst = ps.tile([P, QCH], F32, tag="st", bufs=2,
                                   name="wdst")
                    nc.tensor.matmul(wdst[:], ident[:], wsrc[:],
                                     start=True, stop=True)

            keepwarm(18)

            qT_sb = sb.tile([P, MT, TL], BF, name="qT_sb")
            kT_sb = sb.tile([P, MT, TL], BF, name="kT_sb")
            vT_sb = sb.tile([P, MT, TL], BF, name="vT_sb")
            # v natural layout, per 128-token tile per head: [64 v | 1]
            v_sb = sb.tile([P, NKT, HL, D + 1], BF, name="v_sb")
            nc.gpsimd.memset(v_sb[:], 1.0)

            # partial-O DRAM staging + RS outputs
            po_dram = [
                dram.tile([C, QCH], BF, name=f"po_dram{c}")
                for c in range(NQC)
            ]
            rs_out = [
                dram.tile([DL, QCH], BF, name=f"rs_out{c}")
                for c in range(NQC)
            ]

            with tc.tile_pool(name="xp", bufs=1) as xp:
                xT_sb = xp.tile([P, NCH, TL], BF, name="xT_sb")
                # chunk 0 fast (8 small DMAs on scalar), rest batched
                for ci in range(NCH):
                    nc.scalar.dma_start(
                        xT_sb[:, ci, 0:QCH], xT[ci * P:(ci + 1) * P, 0:QCH]
                    )
                for s0, s1 in ((QCH, 2 * QCH), (2 * QCH, TL)):
                    for ci in range(NCH):
                        nc.gpsimd.dma_start(
                            xT_sb[:, ci, s0:s1], xT[ci * P:(ci + 1) * P, s0:s1]
                        )

                def proj_group(tch, wsb, mt, dst):
                    tsl = slice(tch * QCH, (tch + 1) * QCH)
                    pj = ps.tile([P, QCH], F32, tag="st", bufs=2, name="pj")
                    for ci in range(NCH):
                        nc.tensor.matmul(
                            pj[:], wsb[:, ci, mt * P:(mt + 1) * P],
                            xT_sb[:, ci, tsl],
                            start=(ci == 0), stop=(ci == NCH - 1),
                        )
                    nc.vector.tensor_copy(dst[:, mt, tsl], pj[:])

                def vtrans(t16, mt):
                    tr = ps.tile([P, P], BF, tag="st", bufs=2, name="tr")
                    nc.tensor.transpose(
                        tr[:], vT_sb[:, mt, t16 * P:(t16 + 1) * P], ident[:]
                    )
                    out_ap = v_sb[:, t16, 2 * mt:2 * mt + 2, 0:D]
                    in_ap = tr[:].rearrange("p (h x) -> p h x", h=2)
                    nc.vector.tensor_copy(out_ap, in_ap)

                ych_tiles = {}

                def attn_head(jq, h, lookahead=True):
                    """Score+exp+PV for one head of one 512-token block.

                    Scores for pair p+1 are issued before the PVs of
                    pair p, so the PE isn't gated on exp(p)."""
                    mt, hh = h // 2, h % 2
                    rsl = slice(hh * D, (hh + 1) * D)
                    q0 = jq * QCH
                    yt = ps.tile([D + 1, QCH], F32, tag="yt", bufs=2,
                                 name="yt")
                    nkt = 4 * jq + 4
                    npr = nkt // 2

                    def scores(pr):
                        st = ps.tile([P, 2 * QCH], F32, tag="st", bufs=2,
                                     name="st")
                        pt = sb.tile([P, 2 * QCH], BF, tag="pt", bufs=4,
                                     name="pt")
                        for half in range(2):
                            kt = 2 * pr + half
                            k0 = kt * P
                            i = kt - 4 * jq
                            qv = max(i, 0) * P
                            ssl = slice(half * QCH + qv, (half + 1) * QCH)
                            nc.tensor.matmul(
                                st[:, ssl],
                                kT_sb[rsl, mt, k0:k0 + P],
                                qT_sb[rsl, mt, q0 + qv:q0 + QCH],
                                start=True, stop=(i < 0),
                            )
                            if i >= 0:
                                nc.tensor.matmul(
                                    st[:, half * QCH + qv:
                                       half * QCH + qv + P],
                                    ident[:], mneg[:],
                                    start=False, stop=True,
                                )
                        qv0 = max(2 * pr - 4 * jq, 0) * P
                        nc.scalar.activation(
                            pt[:, qv0:], st[:, qv0:], AF.Exp,
                            scale=float(SCALE)
                        )
                        return pt

                    def pv(pr, pt):
                        for half in range(2):
                            kt = 2 * pr + half
                            qv = max(kt - 4 * jq, 0) * P
                            nc.tensor.matmul(
                                yt[:, qv:QCH],
                                v_sb[:, kt, h, :],
                                pt[:, half * QCH + qv:(half + 1) * QCH],
                                start=(kt == 0), stop=(kt == nkt - 1),
                            )

                    if lookahead:
                        pts = [scores(0)]
                        for pr in range(npr):
                            if pr + 1 < npr:
                                pts.append(scores(pr + 1))
                            pv(pr, pts[pr])
                    else:
                        for pr in range(npr):
                            pv(pr, scores(pr))

                    den = sb.tile([1, QCH], F32, tag="den", bufs=4,
                                  name="den")
                    nc.vector.tensor_copy(den[:], yt[D:D + 1, :])
                    return yt, den

                def attn_evict(jq, h, yt, den):
                    mt, hh = h // 2, h % 2
                    bc = sb.tile([D, QCH], F32, tag="bc", bufs=3, name="bc")
                    nc.gpsimd.partition_broadcast(bc[:], den[:])
                    rcp = sb.tile([D, QCH], F32, tag="rcp", bufs=3,
                                  name="rcp")
                    scr = sb.tile([D, QCH], F32, tag="scr", bufs=3,
                                  name="scr")
                    nc.vector.reciprocal_approx_accurate(
                        rcp[:], bc[:], scratch=scr[:]
                    )
                    ych = ych_tiles[jq]
                    nc.vector.tensor_mul(
                        ych[hh * D:(hh + 1) * D, mt, :], yt[0:D, :], rcp[:]
                    )

                def po_group(c):
                    """Partial output projection for token chunk c:
                    po[1024, 512] = WoT_shard @ ych -> DRAM, then RS."""
                    ych = ych_tiles[c]
                    ob = sb.tile([P, NRT, QCH], BF, tag="ob", bufs=2,
                                 name="ob")
                    for rt in range(NRT):
                        po = ps.tile([P, QCH], F32, tag="st", bufs=2,
                                     name="po")
                        for ci in range(MT):
                            nc.tensor.matmul(
                                po[:], wo_sb[:, ci, rt * P:(rt + 1) * P],
                                ych[:, ci, :],
                                start=(ci == 0), stop=(ci == MT - 1),
                            )
                        nc.vector.tensor_copy(ob[:, rt, :], po[:])
                    dview = po_dram[c][:].rearrange("(r p) t -> p r t", p=P)
                    nc.sync.dma_start(dview, ob[:])
                    nc.gpsimd.collective_compute(
                        "ReduceScatter",
                        mybir.AluOpType.add,
                        replica_groups=RGROUPS,
                        ins=[po_dram[c][:]],
                        outs=[rs_out[c][:]],
                    )

                def out_write(c, eng):
                    # deferred to program end: a DMA after the RS trigger
                    # on a compute queue would block that queue (and the
                    # whole pipeline behind it) on RS completion
                    nsp = 2 if c == NQC - 1 else 1
                    w = QCH // nsp
                    for s in range(nsp):
                        o0 = c * QCH + s * w
                        eng.dma_start(
                            out[:, o0:o0 + w],
                            rs_out[c][:, s * w:(s + 1) * w],
                        )

                # ---- filler queue: projection chunks issued JIT ----
                def proj_items(tchs):
                    items = []
                    for tch in tchs:
                        for wsb, dst in ((wq_sb, qT_sb), (wk_sb, kT_sb),
                                         (wv_sb, vT_sb)):
                            for mt in range(MT):
                                items.append((proj_group, (tch, wsb, mt,
                                                           dst)))
                        for t16 in range(tch * 4, tch * 4 + 4):
                            for mt in range(MT):
                                items.append((vtrans, (t16, mt)))
                    return items

                filler = []

                def pop_filler(n):
                    for _ in range(min(n, len(filler))):
                        fn, args = filler.pop(0)
                        fn(*args)

                # chunk 0 projections up front (attention needs them)
                for fn, args in proj_items([0]):
                    fn(*args)

                filler = proj_items([1, 2, 3])

                # ---- attention blocks ----
                for jq in range(NQC):
                    ych_tiles[jq] = sb.tile([P, MT, QCH], BF, tag="ych",
                                            bufs=2, name="ych")
                    for h in range(HL):
                        yt, den = attn_head(jq, h)
                        if jq < NQC - 1:
                            # chunk jq+1 projections as PE filler; spread
                            # across the 4 head gaps (14 items/chunk)
                            pop_filler(4 if h else 3)
                        elif h == 0:
                            pop_filler(99)
                        attn_evict(jq, h, yt, den)
                        if h == 1 and jq > 0:
                            po_group(jq - 1)
                    if jq == NQC - 1:
                        po_group(jq)
                out_write(0, nc.sync)
                out_write(1, nc.gpsimd)
                out_write(2, nc.sync)
                keepwarm(6)
                out_write(3, nc.gpsimd)

    nc.finalize()
    return nc


_GRAPH = None


def _get_graph():
    global _GRAPH
    if _GRAPH is None:
        _GRAPH = build_graph()
    return _GRAPH


def prepare_in_maps(x, Wq, Wk, Wv, Wo):
    x = np.asarray(x, np.float32)
    Wq = np.asarray(Wq, np.float32)
    Wk = np.asarray(Wk, np.float32)
    Wv = np.asarray(Wv, np.float32)
    Wo = np.asarray(Wo, np.float32)

    bf = ml_dtypes.bfloat16
    xTh = [
        np.ascontiguousarray(x[b].T).astype(bf) for b in range(B)
    ]  # [C, TL] each
    in_maps = []
    for r in range(N_CORES):
        b, g = r // GR, r % GR
        sl = slice(g * DL, (g + 1) * DL)
        wqkv = np.empty((P, 3, NCH, DL), np.float32)
        for w, W in enumerate((Wq, Wk, Wv)):
            # shard rows -> [in=(ci p), out] -> [p, ci, m]
            wqkv[:, w] = W[sl].T.reshape(NCH, P, DL).transpose(1, 0, 2)
        # woT[p, ci, rt*128]: out[rt block] += Wo[rt, ci block].T @ y[ci]
        wo = np.ascontiguousarray(Wo[:, sl].T)  # [DL, C] = lhsT
        woT = wo.reshape(MT, P, C).transpose(1, 0, 2)  # [p, ci, C]
        wall = np.concatenate(
            [wqkv.reshape(P, 3 * NCH * DL), woT.reshape(P, MT * C)], axis=1
        )
        in_maps.append({
            "xT": xTh[b],
            "wall": np.ascontiguousarray(wall).astype(bf),
        })
    return in_maps


def assemble_output(results):
    outT = np.empty((B, C, TL), np.float32)
    for r in range(N_CORES):
        b, g = r // GR, r % GR
        outT[b, g * DL:(g + 1) * DL] = np.asarray(
            results[r]["out"], np.float32
        )
    return np.ascontiguousarray(outT.transpose(0, 2, 1))  # [B, T, C]


def kernel(x, Wq, Wk, Wv, Wo):
    nc = _get_graph()
    in_maps = prepare_in_maps(x, Wq, Wk, Wv, Wo)
    res = run_bass_kernel_spmd(nc, in_maps, core_ids=list(range(N_CORES)))
    return assemble_output(res.results)


# revision 17
# speedup vs baseline: 1.1650x; 1.0953x over previous
"""Causal multi-head attention block (B=2, T=2048, C=1024, H=16) on 8 TRN2
NeuronCores.

Sharding: 2D tensor parallel — core r owns batch b = r//4 and head quad
g = r%4 (heads 4g..4g+3, feature slice [256g, 256g+256)). Each core
projects q/k/v for its 256 features over its batch's 2048 tokens
(x^T replicated per batch), runs causal attention for its 4 heads, then
computes a PARTIAL output projection out_partial = Wo[:, 256g:256g+256]
@ y_local — full 1024 output rows, contracting only the local features.
The 4 cores of a batch ReduceScatter(add) the partials so core g ends
with output rows [256g, 256g+256) — 3 MB of wire per core instead of
the ~7 MB an 8-way y-AllGather moves, and the O-projection matmuls are
collective-independent PE filler instead of tail work.

On-device everything is feature-major (transposed) so the TensorEngine
contraction axis sits on SBUF partitions and the softmax denominator
arrives via a ones-column appended to V:

  qT/kT [128, 2, 2048] = W_shard @ x^T           (2 feature tiles)
  v_sb[128tok, kt, h, 0:64] = x_tile^T @ Wv      (v built NATURALLY per
                                                  token tile — no
                                                  transpose pass)
  ST tile [128k, 512q] = kT[64h slice].T @ qT    (contract d=64)
  PT = exp(ST * 1/sqrt(d))                       (no max-subtraction:
                                                  logits ~N(0,1))
  causal mask: PT[diag 128-block] *= tri (upper-triangular 0/1 bf16,
      vector multiply) — cheaper than injecting -1e9 into PSUM via
      matmul, which cost an ident LDWEIGHTS + matmul per diagonal tile
  yT [65, 512] += [v | 1].T @ PT                 (row 64 = softmax denom)
  ych[*, 512]  = yT[0:64] * bcast(recip(yT[64]))
  po [128rt, 512] += WoT[ci, rt] @ ych[:, ci]    (8 row tiles × 2 ci)

Performance structure (from perfetto/HAM analysis): the PE idle-activity
throttle (HAM) halves the clock for any window containing idle time — a
dense-matmul microbench holds 2.4 GHz while the v1 kernel averaged
~0.63×. So the whole kernel is ONE software-pipelined job stream:
score-pair jobs and filler GEMM jobs (projection chunks, partial-O
groups, v tiles) are interleaved, with each pair's PV matmuls deferred
two jobs so the PE never waits on the exp of the pair it just scored.
ReduceScatter chunks fire per 512-token block as soon as the block's
partial-O lands in DRAM; rs_out -> out DMAs all run at program end
(a DMA after an RS trigger on a compute queue would block that queue on
RS completion and stall the pipeline).

Inputs are bf16 (host-side cast); accumulation is f32 in PSUM; the
output shard is written bf16 and upcast to f32 on the host.
"""

import numpy as np
import ml_dtypes

import concourse.bacc as bacc
import concourse.mybir as mybir
import concourse.tile as tile
from concourse.bass_utils import run_bass_kernel_spmd
from concourse.masks import make_identity

N_CORES = 8
B, T, C, H = 2, 2048, 1024, 16
D = 64                # head dim
GR = 4                # head-group cores per batch
HL = 4                # heads per core
DL = HL * D           # local feature dim = 256
MT = DL // 128        # feature tiles per core = 2
TL = T                # local tokens per core = its batch's 2048
P = 128
NCH = C // P          # 8 contraction chunks for q/k/v projections
QCH = 512             # q-chunk / token chunk
NQC = TL // QCH       # 4 chunks
NKT = TL // P         # 16 k-tiles
NRT = C // P          # 8 output row tiles for partial O-proj
SCALE = 1.0 / np.sqrt(D)
LA = 2                # job-stream lookahead (score pairs ahead of PV)

BF = mybir.dt.bfloat16
F32 = mybir.dt.float32
AF = mybir.ActivationFunctionType

RGROUPS = [[0, 1, 2, 3], [4, 5, 6, 7]]


def build_graph():
    nc = bacc.Bacc("TRN2", target_bir_lowering=False, debug=False)

    xT = nc.dram_tensor("xT", [C, TL], BF, kind="ExternalInput")
    # qkv shards [p, w, ci, m(256)] then woT [p, ci2, rt*128]
    WQKV = 3 * NCH * DL
    WO = MT * C
    wall = nc.dram_tensor("wall", [P, WQKV + WO], BF, kind="ExternalInput")
    out = nc.dram_tensor("out", [DL, TL], BF, kind="ExternalOutput")

    with tile.TileContext(nc) as tc:
        with (
            tc.tile_pool(name="sb", bufs=1) as sb,
            tc.tile_pool(name="ps", bufs=1, space="PSUM") as ps,
            tc.tile_pool(name="dram", bufs=1, space="DRAM") as dram,
        ):
            # ---- loads: few big DMAs spread over the 3 DMA queues ----
            w_sb = sb.tile([P, WQKV + WO], BF, name="w_sb")
            WC4 = (WQKV + WO) // 4
            xcis = {nc.scalar: (0, 1, 2), nc.sync: (3, 4, 5),
                    nc.gpsimd: (6, 7)}
            nc.scalar.dma_start(w_sb[:, 0:WC4], wall[:, 0:WC4])
            nc.sync.dma_start(w_sb[:, WC4:2 * WC4], wall[:, WC4:2 * WC4])
            nc.gpsimd.dma_start(w_sb[:, 2 * WC4:3 * WC4],
                                wall[:, 2 * WC4:3 * WC4])
            nc.gpsimd.dma_start(w_sb[:, 3 * WC4:], wall[:, 3 * WC4:])

            xT_sb = sb.tile([P, NCH, TL], BF, name="xT_sb")
            for eng, cis in xcis.items():
                for ci in cis:
                    eng.dma_start(xT_sb[:, ci, :],
                                  xT[ci * P:(ci + 1) * P, :])

            w3 = w_sb[:, 0:WQKV].rearrange("p (w a m) -> p w a m", w=3, a=NCH)
            wq_sb, wk_sb, wv_sb = (w3[:, i] for i in range(3))
            wo_sb = w_sb[:, WQKV:].rearrange("p (a r) -> p a r", a=MT)

            ident = sb.tile([P, P], BF, name="ident")
            make_identity(nc, ident)
            # upper-triangular (q >= k) 0/1 mask for diagonal blocks
            tri = sb.tile([P, P], BF, name="tri")
            nc.gpsimd.memset(tri[:], 1.0)
            nc.gpsimd.affine_select(
                out=tri[:], in_=tri[:],
                compare_op=mybir.AluOpType.is_ge,
                fill=0.0, base=0, channel_multiplier=-1, pattern=[[1, P]],
            )
            wsrc = sb.tile([P, QCH], BF, name="wsrc")
            nc.vector.memset(wsrc[:], 0.5)

            def keepwarm(n):
                for _ in range(n):
                    wdst = ps.tile([P, QCH], F32, tag="st", bufs=3,
                                   name="wdst")
                    nc.tensor.matmul(wdst[:], ident[:], wsrc[:],
                                     start=True, stop=True)

            keepwarm(14)

            qT_sb = sb.tile([P, MT, TL], BF, name="qT_sb")
            kT_sb = sb.tile([P, MT, TL], BF, name="kT_sb")
            # v natural layout, per 128-token tile per head: [64 v | 1]
            v_sb = sb.tile([P, NKT, HL, D + 1], BF, name="v_sb")
            nc.gpsimd.memset(v_sb[:], 1.0)

            po_dram = [
                dram.tile([C, QCH], BF, name=f"po_dram{c}")
                for c in range(NQC)
            ]
            rs_out = [
                dram.tile([DL, QCH], BF, name=f"rs_out{c}")
                for c in range(NQC)
            ]

            # ---------- job bodies ----------
            def proj_pair(pch, wsb, mt, dst):
                # two 512-wide accumulation groups, one 1024-wide evict
                t0 = pch * 2 * QCH
                pj = ps.tile([P, 2 * QCH], F32, tag="st", bufs=3, name="pj")
                for half in range(2):
                    hsl = slice(t0 + half * QCH, t0 + (half + 1) * QCH)
                    for ci in range(NCH):
                        nc.tensor.matmul(
                            pj[:, half * QCH:(half + 1) * QCH],
                            wsb[:, ci, mt * P:(mt + 1) * P],
                            xT_sb[:, ci, hsl],
                            start=(ci == 0), stop=(ci == NCH - 1),
                        )
                nc.vector.tensor_copy(
                    dst[:, mt, t0:t0 + 2 * QCH], pj[:]
                )

            def v_tile(t16):
                # v_nat [128 tok, 256 feat] = x_tile^T @ Wv
                vps = ps.tile([P, DL], F32, tag="st", bufs=3, name="vps")
                for ci in range(NCH):
                    nc.tensor.matmul(
                        vps[:], xT_sb[:, ci, t16 * P:(t16 + 1) * P],
                        wv_sb[:, ci, :],
                        start=(ci == 0), stop=(ci == NCH - 1),
                    )
                nc.vector.tensor_copy(
                    v_sb[:, t16, :, 0:D],
                    vps[:].rearrange("p (h x) -> p h x", h=HL),
                )

            ych_tiles = {}
            yt_tiles = {}

            def scores(jq, h, pr):
                """Scores + exp (+ causal mask) for one k-tile pair."""
                mt, hh = h // 2, h % 2
                rsl = slice(hh * D, (hh + 1) * D)
                q0 = jq * QCH
                st = ps.tile([P, 2 * QCH], F32, tag="st", bufs=3, name="st")
                pt = sb.tile([P, 2 * QCH], BF, tag="pt", bufs=5, name="pt")
                diag = []
                for half in range(2):
                    kt = 2 * pr + half
                    i = kt - 4 * jq
                    qv = max(i, 0) * P
                    nc.tensor.matmul(
                        st[:, half * QCH + qv:(half + 1) * QCH],
                        kT_sb[rsl, mt, kt * P:(kt + 1) * P],
                        qT_sb[rsl, mt, q0 + qv:q0 + QCH],
                        start=True, stop=True,
                    )
                    if i >= 0:
                        diag.append(half * QCH + qv)
                qv0 = max(2 * pr - 4 * jq, 0) * P
                nc.scalar.activation(
                    pt[:, qv0:], st[:, qv0:], AF.Exp, scale=float(SCALE)
                )
                for c0 in diag:
                    nc.vector.tensor_mul(
                        pt[:, c0:c0 + P], pt[:, c0:c0 + P], tri[:]
                    )
                return pt

            def pv(jq, h, pr, pt):
                yt = yt_tiles.get((jq, h))
                if yt is None:
                    yt = yt_tiles[(jq, h)] = ps.tile(
                        [D + 1, QCH], F32, tag="yt", bufs=2, name="yt"
                    )
                nkt = 4 * jq + 4
                for half in range(2):
                    kt = 2 * pr + half
                    qv = max(kt - 4 * jq, 0) * P
                    nc.tensor.matmul(
                        yt[:, qv:QCH],
                        v_sb[:, kt, h, :],
                        pt[:, half * QCH + qv:(half + 1) * QCH],
                        start=(kt == 0), stop=(kt == nkt - 1),
                    )

            def evict(jq, h):
                yt = yt_tiles.pop((jq, h))
                den = sb.tile([1, QCH], F32, tag="den", bufs=4, name="den")
                nc.vector.tensor_copy(den[:], yt[D:D + 1, :])
                bc = sb.tile([D, QCH], F32, tag="bc", bufs=3, name="bc")
                nc.gpsimd.partition_broadcast(bc[:], den[:])
                rcp = sb.tile([D, QCH], F32, tag="rcp", bufs=3, name="rcp")
                scr = sb.tile([D, QCH], F32, tag="scr", bufs=3, name="scr")
                nc.vector.reciprocal_approx_accurate(
                    rcp[:], bc[:], scratch=scr[:]
                )
                mt, hh = h // 2, h % 2
                ych = ych_tiles[jq]
                nc.vector.tensor_mul(
                    ych[hh * D:(hh + 1) * D, mt, :], yt[0:D, :], rcp[:]
                )

            def po_group(c):
                """Partial O-proj for chunk c -> DRAM -> RS trigger."""
                ych = ych_tiles.pop(c)
                ob = sb.tile([P, NRT, QCH], BF, tag="ob", bufs=2, name="ob")
                for rt in range(NRT):
                    po = ps.tile([P, QCH], F32, tag="st", bufs=3, name="po")
                    for ci in range(MT):
                        nc.tensor.matmul(
                            po[:], wo_sb[:, ci, rt * P:(rt + 1) * P],
                            ych[:, ci, :],
                            start=(ci == 0), stop=(ci == MT - 1),
                        )
                    nc.vector.tensor_copy(ob[:, rt, :], po[:])
                dview = po_dram[c][:].rearrange("(r p) t -> p r t", p=P)
                nc.sync.dma_start(dview, ob[:])
                nc.gpsimd.collective_compute(
                    "ReduceScatter",
                    mybir.AluOpType.add,
                    replica_groups=RGROUPS,
                    ins=[po_dram[c][:]],
                    outs=[rs_out[c][:]],
                )

            # ---------- job stream ----------
            # each job: (phase1, phase2) — phase2 runs LA jobs later
            jobs = []

            def add_filler(fn, *args):
                jobs.append((lambda a=args: fn(*a), None))

            def add_pair(jq, h, pr, last):
                def p1(a=(jq, h, pr)):
                    return scores(*a)

                def p2(pt, a=(jq, h, pr), last=last):
                    pv(*a, pt)
                    if last:
                        evict(a[0], a[1])
                jobs.append((p1, p2))

            # chunks 0+1 projections (immediate: block 0 needs them)
            for wsb, dst in ((wq_sb, qT_sb), (wk_sb, kT_sb)):
                for mt in range(MT):
                    add_filler(proj_pair, 0, wsb, mt, dst)
            for t16 in range(4):
                add_filler(v_tile, t16)

            # blocks with fillers woven in
            for jq in range(NQC):
                npr = (4 * jq + 4) // 2
                for h in range(HL):
                    for pr in range(npr):
                        add_pair(jq, h, pr, pr == npr - 1)
                    # weave fillers after each head
                    if jq == 0:
                        if h == 0:
                            for t16 in range(4, 8):
                                add_filler(v_tile, t16)
                        elif h == 1:
                            for wsb, dst in ((wq_sb, qT_sb), (wk_sb, kT_sb)):
                                for mt in range(MT):
                                    add_filler(proj_pair, 1, wsb, mt, dst)
                        elif h == 2:
                            for t16 in range(8, 12):
                                add_filler(v_tile, t16)
                        else:
                            for t16 in range(12, 16):
                                add_filler(v_tile, t16)
                    elif jq > 0 and h == 0:
                        add_filler(po_group, jq - 1)
            # flush the pipeline (phase2 lags by LA) before the last po
            for _ in range(LA):
                add_filler(keepwarm, 1)
            add_filler(po_group, NQC - 1)

            # ---------- software-pipelined emission ----------
            for jq in range(NQC):
                ych_tiles[jq] = sb.tile([P, MT, QCH], BF, tag="ych",
                                        bufs=2, name="ych")

            pending = []
            for i in range(len(jobs) + LA):
                if i < len(jobs):
                    p1, p2 = jobs[i]
                    r = p1()
                    pending.append((p2, r))
                if i >= LA:
                    p2, r = pending[i - LA]
                    if p2 is not None:
                        p2(r)

            out_write = [
                (0, nc.sync), (1, nc.gpsimd), (2, nc.sync), (3, nc.gpsimd)
            ]
            for c, eng in out_write:
                nsp = 2 if c == NQC - 1 else 1
                w = QCH // nsp
                for s in range(nsp):
                    o0 = c * QCH + s * w
                    eng.dma_start(
                        out[:, o0:o0 + w], rs_out[c][:, s * w:(s + 1) * w]
                    )

    nc.finalize()
    return nc


# pv() needs yt allocated; allocate inside pv via yt_tiles guard
_GRAPH = None


def _get_graph():
    global _GRAPH
    if _GRAPH is None:
        _GRAPH = build_graph()
    return _GRAPH


def prepare_in_maps(x, Wq, Wk, Wv, Wo):
    x = np.asarray(x, np.float32)
    Wq = np.asarray(Wq, np.float32)
    Wk = np.asarray(Wk, np.float32)
    Wv = np.asarray(Wv, np.float32)
    Wo = np.asarray(Wo, np.float32)

    bf = ml_dtypes.bfloat16
    xTh = [np.ascontiguousarray(x[b].T).astype(bf) for b in range(B)]
    in_maps = []
    for r in range(N_CORES):
        b, g = r // GR, r % GR
        sl = slice(g * DL, (g + 1) * DL)
        wqkv = np.empty((P, 3, NCH, DL), np.float32)
        for w, W in enumerate((Wq, Wk, Wv)):
            wqkv[:, w] = W[sl].T.reshape(NCH, P, DL).transpose(1, 0, 2)
        wo = np.ascontiguousarray(Wo[:, sl].T)  # [DL, C] = lhsT
        woT = wo.reshape(MT, P, C).transpose(1, 0, 2)  # [p, ci, C]
        wall = np.concatenate(
            [wqkv.reshape(P, 3 * NCH * DL), woT.reshape(P, MT * C)], axis=1
        )
        in_maps.append({
            "xT": xTh[b],
            "wall": np.ascontiguousarray(wall).astype(bf),
        })
    return in_maps


def assemble_output(results):
    outT = np.empty((B, C, TL), np.float32)
    for r in range(N_CORES):
        b, g = r // GR, r % GR
        outT[b, g * DL:(g + 1) * DL] = np.asarray(
            results[r]["out"], np.float32
        )
    return np.ascontiguousarray(outT.transpose(0, 2, 1))  # [B, T, C]


def kernel(x, Wq, Wk, Wv, Wo):
    nc = _get_graph()
    in_maps = prepare_in_maps(x, Wq, Wk, Wv, Wo)
    res = run_bass_kernel_spmd(nc, in_maps, core_ids=list(range(N_CORES)))
    return assemble_output(res.results)


# revision 26
# speedup vs baseline: 1.2113x; 1.0397x over previous
"""Causal multi-head attention block (B=2, T=2048, C=1024, H=16) on 8 TRN2
NeuronCores.

Sharding: 2D tensor parallel — core r owns batch b = r//4 and head quad
g = r%4 (heads 4g..4g+3, feature slice [256g, 256g+256)). Each core
projects q/k/v for its 256 features over its batch's 2048 tokens
(x^T replicated per batch), runs causal attention for its 4 heads, then
computes a PARTIAL output projection out_partial = Wo[:, 256g:256g+256]
@ y_local — full 1024 output rows, contracting only the local features.
The 4 cores of a batch ReduceScatter(add) the partials so core g ends
with output rows [256g, 256g+256) — 3 MB of wire per core instead of
the ~7 MB an 8-way y-AllGather moves, and the O-projection matmuls are
collective-independent PE filler instead of tail work.

On-device everything is feature-major (transposed) so the TensorEngine
contraction axis sits on SBUF partitions and the softmax denominator
arrives via a ones-column appended to V:

  qT/kT [128, 2, 2048] = W_shard @ x^T           (2 feature tiles)
  v_sb[128tok, kt, h, 0:64] = x_tile^T @ Wv      (v built NATURALLY per
                                                  token tile — no
                                                  transpose pass)
  ST tile [128k, 512q] = kT[64h slice].T @ qT    (contract d=64)
  PT = exp(ST * 1/sqrt(d))                       (no max-subtraction:
                                                  logits ~N(0,1))
  causal mask: PT[diag 128-block] *= tri (upper-triangular 0/1 bf16,
      vector multiply) — cheaper than injecting -1e9 into PSUM via
      matmul, which cost an ident LDWEIGHTS + matmul per diagonal tile
  yT [65, 512] += [v | 1].T @ PT                 (row 64 = softmax denom)
  ych[*, 512]  = yT[0:64] * bcast(recip(yT[64]))
  po [128rt, 512] += WoT[ci, rt] @ ych[:, ci]    (8 row tiles × 2 ci)

Performance structure (from perfetto/HAM analysis): the PE idle-activity
throttle (HAM) halves the clock for any window containing idle time — a
dense-matmul microbench holds 2.4 GHz while the v1 kernel averaged
~0.63×. So the whole kernel is ONE software-pipelined job stream:
score-pair jobs and filler GEMM jobs (projection chunks, partial-O
groups, v tiles) are interleaved, with each pair's PV matmuls deferred
two jobs so the PE never waits on the exp of the pair it just scored.
ReduceScatter chunks fire per 512-token block as soon as the block's
partial-O lands in DRAM; rs_out -> out DMAs all run at program end
(a DMA after an RS trigger on a compute queue would block that queue on
RS completion and stall the pipeline).

Inputs are bf16 (host-side cast); accumulation is f32 in PSUM; the
output shard is written bf16 and upcast to f32 on the host.
"""

import numpy as np
import ml_dtypes

import concourse.bacc as bacc
import concourse.mybir as mybir
import concourse.tile as tile
from concourse.bass_utils import run_bass_kernel_spmd
from concourse.masks import make_identity

N_CORES = 8
B, T, C, H = 2, 2048, 1024, 16
D = 64                # head dim
GR = 4                # head-group cores per batch
HL = 4                # heads per core
DL = HL * D           # local feature dim = 256
MT = DL // 128        # feature tiles per core = 2
TL = T                # local tokens per core = its batch's 2048
P = 128
NCH = C // P          # 8 contraction chunks for q/k/v projections
QCH = 512             # q-chunk / token chunk
NQC = TL // QCH       # 4 chunks
NKT = TL // P         # 16 k-tiles
NRT = C // P          # 8 output row tiles for partial O-proj
SCALE = 1.0 / np.sqrt(D)
LA = 2                # job-stream lookahead (score pairs ahead of PV)

BF = mybir.dt.bfloat16
F32 = mybir.dt.float32
AF = mybir.ActivationFunctionType

RGROUPS = [[0, 1, 2, 3], [4, 5, 6, 7]]


def build_graph():
    nc = bacc.Bacc("TRN2", target_bir_lowering=False, debug=False)

    xT = nc.dram_tensor("xT", [C, TL], BF, kind="ExternalInput")
    # qkv shards [p, w, ci, m(256)] then woT [p, ci2, rt*128]
    WQKV = 3 * NCH * DL
    WO = MT * C
    wall = nc.dram_tensor("wall", [P, WQKV + WO], BF, kind="ExternalInput")
    out = nc.dram_tensor("out", [DL, TL], BF, kind="ExternalOutput")

    with tile.TileContext(nc) as tc:
        with (
            tc.tile_pool(name="sb", bufs=1) as sb,
            tc.tile_pool(name="ps", bufs=1, space="PSUM") as ps,
            tc.tile_pool(name="dram", bufs=1, space="DRAM") as dram,
        ):
            # ---- loads: few big DMAs spread over the 3 DMA queues ----
            # wq (chunk 0) first on scalar: the first projection LDW
            # gates on it; x spread ci-major so early ci land first
            w_sb = sb.tile([P, WQKV + WO], BF, name="w_sb")
            WC4 = (WQKV + WO) // 4
            xT_sb = sb.tile([P, NCH, TL], BF, name="xT_sb")

            def xdma(eng, ci):
                eng.dma_start(xT_sb[:, ci, :], xT[ci * P:(ci + 1) * P, :])

            def wdma(eng, i):
                eng.dma_start(w_sb[:, i * WC4:(i + 1) * WC4],
                              wall[:, i * WC4:(i + 1) * WC4])

            wdma(nc.scalar, 0)   # wq
            wdma(nc.scalar, 1)   # wk
            for ci in (0, 1, 2):
                xdma(nc.scalar, ci)
            for ci in (3, 4, 5):
                xdma(nc.sync, ci)
            wdma(nc.sync, 2)     # wv
            for ci in (6, 7):
                xdma(nc.gpsimd, ci)
            wdma(nc.gpsimd, 3)   # wo

            w3 = w_sb[:, 0:WQKV].rearrange("p (w a m) -> p w a m", w=3, a=NCH)
            wq_sb, wk_sb, wv_sb = (w3[:, i] for i in range(3))
            wo_sb = w_sb[:, WQKV:].rearrange("p (a r) -> p a r", a=MT)

            ident = sb.tile([P, P], BF, name="ident")
            make_identity(nc, ident)
            # upper-triangular (q >= k) 0/1 mask for diagonal blocks
            tri = sb.tile([P, P], BF, name="tri")
            nc.gpsimd.memset(tri[:], 1.0)
            nc.gpsimd.affine_select(
                out=tri[:], in_=tri[:],
                compare_op=mybir.AluOpType.is_ge,
                fill=0.0, base=0, channel_multiplier=-1, pattern=[[1, P]],
            )
            wsrc = sb.tile([P, QCH], BF, name="wsrc")
            nc.vector.memset(wsrc[:], 0.5)

            def keepwarm(n):
                for _ in range(n):
                    wdst = ps.tile([P, QCH], F32, tag="st", bufs=3,
                                   name="wdst")
                    nc.tensor.matmul(wdst[:], ident[:], wsrc[:],
                                     start=True, stop=True)

            keepwarm(20)

            qT_sb = sb.tile([P, MT, TL], BF, name="qT_sb")
            kT_sb = sb.tile([P, MT, TL], BF, name="kT_sb")
            # v natural layout, per 128-token tile per head: [64 v | 1]
            v_sb = sb.tile([P, NKT, HL, D + 1], BF, name="v_sb")
            nc.gpsimd.memset(v_sb[:], 1.0)

            # blocks: (q0, qw, nkt). The last 512-token block is split
            # into two 256-wide halves so its first RS overlaps the
            # remaining attention and the tail RS is half-size.
            BLOCKS = [
                (0, QCH, 4), (QCH, QCH, 8), (2 * QCH, QCH, 12),
                (3 * QCH, QCH // 2, 14), (3 * QCH + QCH // 2, QCH // 2, 16),
            ]
            NBLK = len(BLOCKS)

            po_dram = [
                dram.tile([C, qw], BF, name=f"po_dram{c}")
                for c, (q0, qw, nkt) in enumerate(BLOCKS)
            ]
            rs_out = [
                dram.tile([DL, qw], BF, name=f"rs_out{c}")
                for c, (q0, qw, nkt) in enumerate(BLOCKS)
            ]

            # ---------- job bodies ----------
            def proj_pair(pch, wsb, mt, dst):
                # two 512-wide accumulation groups, one 1024-wide evict
                t0 = pch * 2 * QCH
                pj = ps.tile([P, 2 * QCH], F32, tag="st", bufs=3, name="pj")
                for half in range(2):
                    hsl = slice(t0 + half * QCH, t0 + (half + 1) * QCH)
                    for ci in range(NCH):
                        nc.tensor.matmul(
                            pj[:, half * QCH:(half + 1) * QCH],
                            wsb[:, ci, mt * P:(mt + 1) * P],
                            xT_sb[:, ci, hsl],
                            start=(ci == 0), stop=(ci == NCH - 1),
                        )
                nc.vector.tensor_copy(
                    dst[:, mt, t0:t0 + 2 * QCH], pj[:]
                )

            def v_tile(t16):
                # v_nat [128 tok, 256 feat] = x_tile^T @ Wv
                vps = ps.tile([P, DL], F32, tag="st", bufs=3, name="vps")
                for ci in range(NCH):
                    nc.tensor.matmul(
                        vps[:], xT_sb[:, ci, t16 * P:(t16 + 1) * P],
                        wv_sb[:, ci, :],
                        start=(ci == 0), stop=(ci == NCH - 1),
                    )
                nc.vector.tensor_copy(
                    v_sb[:, t16, :, 0:D],
                    vps[:].rearrange("p (h x) -> p h x", h=HL),
                )

            ych_tiles = {}
            yt_tiles = {}

            def scores(blk, h, pr):
                """Scores + exp (+ causal mask) for one k-tile pair."""
                q0, qw, nkt = BLOCKS[blk]
                mt, hh = h // 2, h % 2
                rsl = slice(hh * D, (hh + 1) * D)
                st = ps.tile([P, 2 * qw], F32, tag="st", bufs=3, name="st")
                pt = sb.tile([P, 2 * qw], BF, tag="pt", bufs=5, name="pt")
                diag = []
                for half in range(2):
                    kt = 2 * pr + half
                    qv = max(kt * P - q0, 0)
                    nc.tensor.matmul(
                        st[:, half * qw + qv:(half + 1) * qw],
                        kT_sb[rsl, mt, kt * P:(kt + 1) * P],
                        qT_sb[rsl, mt, q0 + qv:q0 + qw],
                        start=True, stop=True,
                    )
                    if kt * P >= q0:
                        diag.append(half * qw + qv)
                qv0 = max(2 * pr * P - q0, 0)
                nc.scalar.activation(
                    pt[:, qv0:], st[:, qv0:], AF.Exp, scale=float(SCALE)
                )
                for c0 in diag:
                    nc.vector.tensor_mul(
                        pt[:, c0:c0 + P], pt[:, c0:c0 + P], tri[:]
                    )
                return pt

            def pv(blk, h, pr, pt):
                q0, qw, nkt = BLOCKS[blk]
                yt = yt_tiles.get((blk, h))
                if yt is None:
                    yt = yt_tiles[(blk, h)] = ps.tile(
                        [D + 1, qw], F32, tag="yt", bufs=2, name="yt"
                    )
                for half in range(2):
                    kt = 2 * pr + half
                    qv = max(kt * P - q0, 0)
                    nc.tensor.matmul(
                        yt[:, qv:qw],
                        v_sb[:, kt, h, :],
                        pt[:, half * qw + qv:(half + 1) * qw],
                        start=(kt == 0), stop=(kt == nkt - 1),
                    )

            def evict(blk, h):
                q0, qw, nkt = BLOCKS[blk]
                yt = yt_tiles.pop((blk, h))
                den = sb.tile([1, qw], F32, tag="den", bufs=4, name="den")
                nc.vector.tensor_copy(den[:], yt[D:D + 1, :])
                bc = sb.tile([D, qw], F32, tag="bc", bufs=3, name="bc")
                nc.gpsimd.partition_broadcast(bc[:], den[:])
                rcp = sb.tile([D, qw], F32, tag="rcp", bufs=3, name="rcp")
                scr = sb.tile([D, qw], F32, tag="scr", bufs=3, name="scr")
                nc.vector.reciprocal_approx_accurate(
                    rcp[:], bc[:], scratch=scr[:]
                )
                mt, hh = h // 2, h % 2
                ych = ych_tiles[blk]
                nc.vector.tensor_mul(
                    ych[hh * D:(hh + 1) * D, mt, :], yt[0:D, :], rcp[:]
                )

            def po_group(c):
                """Partial O-proj for block c -> DRAM -> RS trigger."""
                q0, qw, nkt = BLOCKS[c]
                ych = ych_tiles.pop(c)
                ob = sb.tile([P, NRT, qw], BF, tag="ob", bufs=2, name="ob")
                for rt in range(NRT):
                    po = ps.tile([P, qw], F32, tag="st", bufs=3, name="po")
                    for ci in range(MT):
                        nc.tensor.matmul(
                            po[:], wo_sb[:, ci, rt * P:(rt + 1) * P],
                            ych[:, ci, :],
                            start=(ci == 0), stop=(ci == MT - 1),
                        )
                    nc.vector.tensor_copy(ob[:, rt, :], po[:])
                dview = po_dram[c][:].rearrange("(r p) t -> p r t", p=P)
                nc.sync.dma_start(dview, ob[:])
                nc.gpsimd.collective_compute(
                    "ReduceScatter",
                    mybir.AluOpType.add,
                    replica_groups=RGROUPS,
                    ins=[po_dram[c][:]],
                    outs=[rs_out[c][:]],
                )

            # ---------- job stream ----------
            # each job: (phase1, phase2) — phase2 runs LA jobs later
            jobs = []

            def add_filler(fn, *args):
                jobs.append((lambda a=args: fn(*a), None))

            def add_pair(blk, h, pr, last):
                def p1(a=(blk, h, pr)):
                    return scores(*a)

                def p2(pt, a=(blk, h, pr), last=last):
                    pv(*a, pt)
                    if last:
                        evict(a[0], a[1])
                jobs.append((p1, p2))

            # chunks 0+1 projections (immediate: block 0 needs them)
            for wsb, dst in ((wq_sb, qT_sb), (wk_sb, kT_sb)):
                for mt in range(MT):
                    add_filler(proj_pair, 0, wsb, mt, dst)
            for t16 in range(4):
                add_filler(v_tile, t16)

            # blocks with fillers woven in
            for blk, (q0, qw, nkt) in enumerate(BLOCKS):
                npr = nkt // 2
                for h in range(HL):
                    for pr in range(npr):
                        add_pair(blk, h, pr, pr == npr - 1)
                    # weave fillers after each head
                    if blk == 0:
                        if h == 0:
                            for t16 in range(4, 8):
                                add_filler(v_tile, t16)
                        elif h == 1:
                            for wsb, dst in ((wq_sb, qT_sb), (wk_sb, kT_sb)):
                                for mt in range(MT):
                                    add_filler(proj_pair, 1, wsb, mt, dst)
                        elif h == 2:
                            for t16 in range(8, 12):
                                add_filler(v_tile, t16)
                        else:
                            for t16 in range(12, 16):
                                add_filler(v_tile, t16)
                    elif blk > 0 and h == 0:
                        add_filler(po_group, blk - 1)
            # flush the pipeline (phase2 lags by LA) before the last po
            for _ in range(LA):
                add_filler(keepwarm, 1)
            add_filler(po_group, NBLK - 1)

            # ---------- software-pipelined emission ----------
            for blk, (q0, qw, nkt) in enumerate(BLOCKS):
                ych_tiles[blk] = sb.tile([P, MT, qw], BF, tag="ych",
                                         bufs=2, name="ych")

            pending = []
            for i in range(len(jobs) + LA):
                if i < len(jobs):
                    p1, p2 = jobs[i]
                    r = p1()
                    pending.append((p2, r))
                if i >= LA:
                    p2, r = pending[i - LA]
                    if p2 is not None:
                        p2(r)

            engs = [nc.sync, nc.gpsimd]
            for c, (q0, qw, nkt) in enumerate(BLOCKS):
                engs[c % 2].dma_start(out[:, q0:q0 + qw], rs_out[c][:])

    nc.finalize()
    return nc


# pv() needs yt allocated; allocate inside pv via yt_tiles guard
_GRAPH = None


def _get_graph():
    global _GRAPH
    if _GRAPH is None:
        _GRAPH = build_graph()
    return _GRAPH


def prepare_in_maps(x, Wq, Wk, Wv, Wo):
    x = np.asarray(x, np.float32)
    Wq = np.asarray(Wq, np.float32)
    Wk = np.asarray(Wk, np.float32)
    Wv = np.asarray(Wv, np.float32)
    Wo = np.asarray(Wo, np.float32)

    bf = ml_dtypes.bfloat16
    xTh = [np.ascontiguousarray(x[b].T).astype(bf) for b in range(B)]
    in_maps = []
    for r in range(N_CORES):
        b, g = r // GR, r % GR
        sl = slice(g * DL, (g + 1) * DL)
        wqkv = np.empty((P, 3, NCH, DL), np.float32)
        for w, W in enumerate((Wq, Wk, Wv)):
            wqkv[:, w] = W[sl].T.reshape(NCH, P, DL).transpose(1, 0, 2)
        wo = np.ascontiguousarray(Wo[:, sl].T)  # [DL, C] = lhsT
        woT = wo.reshape(MT, P, C).transpose(1, 0, 2)  # [p, ci, C]
        wall = np.concatenate(
            [wqkv.reshape(P, 3 * NCH * DL), woT.reshape(P, MT * C)], axis=1
        )
        in_maps.append({
            "xT": xTh[b],
            "wall": np.ascontiguousarray(wall).astype(bf),
        })
    return in_maps


def assemble_output(results):
    outT = np.empty((B, C, TL), np.float32)
    for r in range(N_CORES):
        b, g = r // GR, r % GR
        outT[b, g * DL:(g + 1) * DL] = np.asarray(
            results[r]["out"], np.float32
        )
    return np.ascontiguousarray(outT.transpose(0, 2, 1))  # [B, T, C]


def kernel(x, Wq, Wk, Wv, Wo):
    nc = _get_graph()
    in_maps = prepare_in_maps(x, Wq, Wk, Wv, Wo)
    res = run_bass_kernel_spmd(nc, in_maps, core_ids=list(range(N_CORES)))
    return assemble_output(res.results)
